# revision 12
# baseline (speedup 1.0000x reference)
"""Trainium2 Bass kernel for agent attention (sparse_attention problem).

Per-core work (data-parallel over batch B=8 across 8 NeuronCores):
  x[b] [256, 64, 64] -> qkv 3x3 conv (dif-conv + BN folded into weights)
  -> agent attention (8 heads, d=32, 64 agent tokens)
  -> depthwise 3x3 pe conv on v -> 1x1 proj.

v5: 1-D Winograd F(2,3) along W for the qkv conv (6 MACs/output instead
of 9): DVE builds 4 transformed input planes U[j] (even/odd column
combinations, padded rows), PE accumulates P[j] = sum_{ky,kc}
What[j,ky,kc]^T U[j] per 16-row chunk into 4 PSUM banks, and a DVE
"step-2" combines P0..P3 into the two output column parities with the
bias folded in (writes q/k/v directly - no separate evacuation).
q/k/v/att_out live in a column-parity-interleaved layout [oc, b, r, q];
all downstream consumers use matching access patterns, and the proj
epilogue re-interleaves pixels on GpSimd before contiguous output DMA.
Depthwise pe conv runs on GpSimd. v^T via batched XBAR DMA transposes.
"""
import numpy as np

NUM_HEADS = 8
AGENT_NUM = 64
THETA = 0.7
C = 256
H = W = 64
HW = H * W
D = C // NUM_HEADS          # 32
N_CORES = 8
B = 8
PS = 8

_cache = {}


def _build():
    import concourse.bass as bass
    import concourse.tile as tile
    from concourse import bacc, mybir

    f32 = mybir.dt.float32
    bf16 = mybir.dt.bfloat16
    AF = mybir.ActivationFunctionType
    ALU = mybir.AluOpType
    AX = mybir.AxisListType

    nc = bacc.Bacc("TRN2", target_bir_lowering=False, debug=False,
                   enable_asserts=True, num_devices=N_CORES)

    X = nc.dram_tensor("x", [2, 128, HW], bf16, kind="ExternalInput").ap()
    # WQW[mc, p, j, ky, kc, o'] = What[j, ky][128*mc+o', 128*kc+p]
    WQW = nc.dram_tensor("wqw", [6, 128, 4, 3, 2, 128], bf16,
                         kind="ExternalInput").ap()
    BQ = nc.dram_tensor("bq", [128, 6], f32, kind="ExternalInput").ap()
    PEW = nc.dram_tensor("pew", [128, 2, 9], f32, kind="ExternalInput").ap()
    PW = nc.dram_tensor("pw", [128, 2 * 256], bf16, kind="ExternalInput").ap()
    PB = nc.dram_tensor("pb", [128, 2], f32, kind="ExternalInput").ap()
    OUT = nc.dram_tensor("out", [2, 128, HW], bf16, kind="ExternalOutput").ap()

    # softmax exp scale: d^-0.5, with the 1/64 agent-pool mean folded in
    SCALE = (D ** -0.5) / (PS * PS)

    with tile.TileContext(nc) as tc:
        from contextlib import ExitStack
        with ExitStack() as top:
            pers = top.enter_context(tc.tile_pool(name="pers", bufs=1))
            x_sb = [pers.tile([128, HW], bf16, tag=f"x{i}", name=f"x{i}")
                    for i in range(2)]
            # shuffled layout [oc, b(2), r(64), q(32)]: pixel (r, 2q+b)
            q_sb = [pers.tile([128, HW], bf16, tag=f"q{i}", name=f"q{i}")
                    for i in range(2)]
            k_sb = [pers.tile([128, HW], bf16, tag=f"k{i}", name=f"k{i}")
                    for i in range(2)]
            v_sb = [pers.tile([128, HW], bf16, tag=f"v{i}", name=f"v{i}")
                    for i in range(2)]
            att_out = [pers.tile([128, HW], bf16, tag=f"ao{i}", name=f"ao{i}")
                       for i in range(2)]
            # U[j][kc]: transformed input planes [128, 66 rows, 32 q]
            u_t = [[pers.tile([128, 66, 32], bf16, tag=f"u{j}{kc}",
                              name=f"u{j}{kc}") for kc in range(2)]
                   for j in range(4)]
            wq_all = pers.tile([128, 6, 4, 3, 2, 128], bf16, tag="wq",
                               name="wq")
            bq = pers.tile([128, 6], f32, tag="bq", name="bq")
            pew = pers.tile([128, 2, 9], f32, tag="pew", name="pew")
            asum_t = pers.tile([128, 128], f32, tag="asum", name="asum")
            a_sum = [asum_t[:, 64 * i:64 * (i + 1)] for i in range(2)]
            abd_t = pers.tile([128, 512], bf16, tag="abd", name="abd")
            abd = [abd_t[:, 256 * i:256 * (i + 1)] for i in range(2)]
            az_t = pers.tile([128, 4 * 64], bf16, tag="az", name="az")
            attnZ = [az_t[:, 64 * i:64 * (i + 1)] for i in range(4)]
            pw = pers.tile([128, 2 * 256], bf16, tag="pw", name="pwt")
            pb = pers.tile([128, 2], f32, tag="pb", name="pbt")
            hsel = pers.tile([128, 64], bf16, tag="hsel", name="hsel")
            vts_t = pers.tile([128, 32, 260], bf16, tag="vts", name="vts")
            pwv = pw[:].rearrange("p (a b) -> p a b", a=2, b=256)

            nc.vector.memset(hsel[:], 0.0)
            nc.vector.memset(hsel[0:64, 0:32], 1.0)
            nc.vector.memset(hsel[64:128, 32:64], 1.0)
            nc.vector.memset(vts_t[:, :, 64:66], 1.0)
            nc.vector.memset(vts_t[:, :, 194:196], 1.0)
            for j in range(4):
                for kc in range(2):
                    nc.vector.memset(u_t[j][kc][:, 0:1, :], 0.0)
                    nc.vector.memset(u_t[j][kc][:, 65:66, :], 0.0)

            # x first (the input transform gates everything), then weights
            for kc in range(2):
                nc.sync.dma_start(x_sb[kc][:, 0:2048], X[kc, :, 0:2048])
                nc.sync.dma_start(x_sb[kc][:, 2048:4096], X[kc, :, 2048:4096])
            for mc in (4, 5, 0, 1, 2, 3):
                nc.sync.dma_start(wq_all[:, mc], WQW[mc])
            nc.sync.dma_start(bq[:], BQ[:])
            nc.sync.dma_start(pew[:], PEW[:])
            nc.sync.dma_start(pw[:], PW[:])
            nc.sync.dma_start(pb[:], PB[:])

            # ---- input transform: U[j][kc][:, 1+r, q] over x rows r ----
            # xe = x[r, 2q], xo = x[r, 2q+1]
            # U0 = xo[q-1] - xo[q]  (q=0: -xo[0])
            # U1 = xe[q] + xo[q]
            # U2 = xo[q] - xe[q]
            # U3 = xe[q] - xe[q+1]  (q=31: xe[31])
            # row-halved so each half only gates on its x half-DMA
            for rh in range(2):
                for j in range(4):
                    for kc in range(2):
                        xv = x_sb[kc][:].rearrange(
                            "p (r q two) -> p r two q", r=64, q=32, two=2)
                        r0, r1 = 32 * rh, 32 * rh + 32
                        xe = xv[:, r0:r1, 0:1, :]
                        xo = xv[:, r0:r1, 1:2, :]
                        uv = u_t[j][kc][:].rearrange(
                            "p r (one q) -> p r one q", one=1,
                            q=32)[:, 1 + r0:1 + r1]
                        if j == 0:
                            nc.vector.tensor_tensor(
                                uv[:, :, :, 1:32], xo[:, :, :, 0:31],
                                xo[:, :, :, 1:32], ALU.subtract)
                            nc.vector.tensor_scalar_mul(
                                uv[:, :, :, 0:1], xo[:, :, :, 0:1], -1.0)
                        elif j == 1:
                            nc.vector.tensor_tensor(
                                uv, xe, xo, ALU.add)
                        elif j == 2:
                            nc.vector.tensor_tensor(
                                uv, xo, xe, ALU.subtract)
                        else:
                            nc.vector.tensor_tensor(
                                uv[:, :, :, 0:31], xe[:, :, :, 0:31],
                                xe[:, :, :, 1:32], ALU.subtract)
                            nc.vector.tensor_copy(
                                uv[:, :, :, 31:32], xe[:, :, :, 31:32])

            with ExitStack() as ph:
                cps = ph.enter_context(
                    tc.tile_pool(name="cps", bufs=8, space="PSUM"))
                vtt = ph.enter_context(tc.tile_pool(name="vtt", bufs=2))
                stp = ph.enter_context(tc.tile_pool(name="stp", bufs=4))

                # queue of DVE side-work (pe-conv / pooling / abd) thunks,
                # drained a few ops at a time at conv chunk boundaries so
                # the in-order DVE queue never backs up far enough to stall
                # the ACT evacuations (and with them the PSUM rotation)
                dve_q = []

                def drain(n):
                    for _ in range(min(n, len(dve_q))):
                        dve_q.pop(0)()

                # one conv output group: 128 out-channels (mc), shuffled
                # layout; 4 chunks of 16 rows; P[j] accumulated on PE,
                # combined + biased on DVE, written straight to q/k/v
                def conv_group(mc):
                    if mc < 2:
                        dst_t = q_sb[mc]
                    elif mc < 4:
                        dst_t = k_sb[mc - 2]
                    else:
                        dst_t = v_sb[mc - 4]
                    bias = bq[:, mc:mc + 1]
                    for c in range(4):
                        ps = []
                        for j in range(4):
                            p_t = cps.tile([128, 512], f32, tag="cps",
                                           name="cpst")
                            for i, (ky, kc) in enumerate(
                                    (ky, kc) for ky in range(3)
                                    for kc in range(2)):
                                rhs = u_t[j][kc][:, 16 * c + ky:
                                                 16 * c + ky + 16, :]
                                nc.tensor.matmul(
                                    p_t[:], wq_all[:, mc, j, ky, kc, :],
                                    rhs, start=(i == 0), stop=(i == 5))
                            ps.append(p_t)
                        # step-2: b=0 -> P0+P1+P2+bias ; b=1 -> P1-P2-P3+bias
                        # ACT evacuates the four PSUM banks to SBUF bf16
                        # (folding the bias and the P3 sign), then all-SBUF
                        # TT chains run on DVE (b=0) and GpSimd (b=1)
                        p0b = stp.tile([128, 512], bf16, tag="p0b",
                                       name="p0b", bufs=3)
                        nc.scalar.add(p0b[:], ps[0][:], bias)
                        p1e = stp.tile([128, 512], bf16, tag="p1e",
                                       name="p1e", bufs=3)
                        nc.scalar.copy(p1e[:], ps[1][:])
                        p2e = stp.tile([128, 512], bf16, tag="p2e",
                                       name="p2e", bufs=3)
                        nc.scalar.copy(p2e[:], ps[2][:])
                        p3n = stp.tile([128, 512], bf16, tag="p3n",
                                       name="p3n", bufs=3)
                        nc.scalar.activation(p3n[:], ps[3][:], AF.Identity,
                                             bias=bias, scale=-1.0)
                        t0b = stp.tile([128, 512], bf16, tag="t0b",
                                       name="t0b", bufs=2)
                        nc.vector.tensor_tensor(t0b[:], p0b[:], p1e[:],
                                                ALU.add)
                        nc.vector.tensor_tensor(
                            dst_t[:, 512 * c:512 * (c + 1)], t0b[:],
                            p2e[:], ALU.add)
                        t1b = stp.tile([128, 512], bf16, tag="t1b",
                                       name="t1b", bufs=2)
                        nc.gpsimd.tensor_tensor(t1b[:], p1e[:], p2e[:],
                                                ALU.subtract)
                        nc.gpsimd.tensor_tensor(
                            dst_t[:, 2048 + 512 * c:2048 + 512 * (c + 1)],
                            t1b[:], p3n[:], ALU.add)
                        drain(5)

                # depthwise pe conv on GpSimd in the shuffled layout,
                # accumulating into att_out; center tap first (overwrite)
                def pe_conv(cc, g):
                    vv = v_sb[cc][:].rearrange("p (b r q) -> p b r q",
                                               b=2, r=64, q=32)
                    ao = att_out[cc][:].rearrange("p (b r q) -> p b r q",
                                                  b=2, r=64, q=32)
                    g0, g1 = 32 * g, 32 * g + 32
                    for b in range(2):
                        dve_q.append(lambda b=b: nc.vector.tensor_scalar_mul(
                            ao[:, b:b + 1, g0:g1, :],
                            vv[:, b:b + 1, g0:g1, :], pew[:, cc, 4:5]))
                    for s in (0, 1, 2, 3, 5, 6, 7, 8):
                        dy, dx = s // 3 - 1, s % 3 - 1
                        r_lo = max(g0, -dy)
                        r_hi = min(g1, 64 - dy)
                        for b in range(2):
                            if dx == 0:
                                sb, q_lo, q_hi, sq = b, 0, 32, 0
                            elif dx == 1:
                                # src col 2q+b+1
                                if b == 0:
                                    sb, q_lo, q_hi, sq = 1, 0, 32, 0
                                else:
                                    sb, q_lo, q_hi, sq = 0, 0, 31, 1
                            else:
                                # src col 2q+b-1
                                if b == 1:
                                    sb, q_lo, q_hi, sq = 0, 0, 32, 0
                                else:
                                    sb, q_lo, q_hi, sq = 1, 1, 32, -1
                            dst = ao[:, b:b + 1, r_lo:r_hi, q_lo:q_hi]
                            src = vv[:, sb:sb + 1, r_lo + dy:r_hi + dy,
                                     q_lo + sq:q_hi + sq]
                            dve_q.append(
                                lambda dst=dst, src=src, s=s:
                                nc.vector.scalar_tensor_tensor(
                                    dst, src, pew[:, cc, s:s + 1], dst,
                                    ALU.mult, ALU.add))

                # v first
                conv_group(4)
                conv_group(5)

                # v^T via 4 batched XBAR DMA transposes + copies into the
                # interleaved [64ch | ones | 64ch] layout (on ACT engine)
                for cc in range(2):
                    for half in range(2):
                        vt_stg = vtt.tile([128, 32, 64], bf16, tag="vstg",
                                          name="vstg")
                        nc.sync.dma_start_transpose(
                            vt_stg[:],
                            v_sb[cc][64 * half:64 * (half + 1), :])
                        nc.scalar.copy(
                            vts_t[:, :, 130 * cc + 66 * half:
                                  130 * cc + 66 * half + 64],
                            vt_stg[:])

                def pooling(cc):
                    def run(cc=cc):
                        qsrc = q_sb[cc]
                        tmp = stp.tile([128, 2, 8, 8, 8], f32, tag="pool",
                                       name="pool", bufs=1)
                        for b in range(2):
                            qv = qsrc[:, 2048 * b:2048 * (b + 1)].rearrange(
                                "p (by rr bx qq) -> p by rr bx qq",
                                by=8, rr=8, bx=8, qq=4)
                            nc.vector.tensor_reduce(
                                tmp[:, b], qv, AX.X, ALU.add)
                        tv = tmp[:].rearrange("p b by rr bx -> p by bx b rr")
                        nc.vector.tensor_reduce(a_sum[cc], tv, AX.XY,
                                                ALU.add)
                    dve_q.append(run)

                def abd_build():
                    def run():
                        nc.vector.memset(abd_t[:], 0.0)
                        for cc in range(2):
                            for j in range(4):
                                nc.vector.tensor_copy(
                                    abd[cc][32 * j:32 * j + 32,
                                            64 * j:64 * j + 64],
                                    a_sum[cc][32 * j:32 * j + 32, :])
                    dve_q.append(run)

                pe_conv(0, 0)
                pe_conv(0, 1)
                conv_group(0)
                pooling(0)
                pe_conv(1, 0)
                conv_group(1)
                pooling(1)
                abd_build()
                pe_conv(1, 1)
                conv_group(2)
                conv_group(3)
                drain(len(dve_q))

            # ---- stage 1 ----
            with ExitStack() as ph:
                st_ps = ph.enter_context(
                    tc.tile_pool(name="stps", bufs=4, space="PSUM"))
                at_ps = ph.enter_context(
                    tc.tile_pool(name="atps", bufs=4, space="PSUM"))
                etp = ph.enter_context(tc.tile_pool(name="etp", bufs=1))
                attn_ps = [at_ps.tile([128, 66], f32, tag="at", name="at")
                           for _ in range(4)]
                for chp in range(16):   # px-chunk pairs
                    for cc in range(2):
                        sp = st_ps.tile([128, 512], f32, tag="st",
                                        name="stt")
                        for u in range(2):
                            ch = 2 * chp + u
                            nc.tensor.matmul(
                                sp[:, 256 * u:256 * (u + 1)],
                                k_sb[cc][:, 128 * ch:128 * (ch + 1)],
                                abd[cc], start=True, stop=True,
                                skip_group_check=True)
                        et = etp.tile([128, 512], bf16, tag="et",
                                      name="et", bufs=6)
                        nc.scalar.activation(et[:], sp[:], AF.Exp,
                                             scale=SCALE)
                        for u in range(2):
                            ch = 2 * chp + u
                            for half in range(2):
                                hp = 2 * cc + half
                                rhs = vts_t[:, ch, 130 * cc + 64 * half:
                                            130 * cc + 64 * half + 66]
                                nc.tensor.matmul(
                                    attn_ps[hp][:],
                                    et[:, 256 * u + 128 * half:
                                       256 * u + 128 * (half + 1)],
                                    rhs, start=(ch == 0), stop=(ch == 31))

                for hp in range(4):
                    half = hp % 2
                    zc = 64 if half == 0 else 0
                    och = 0 if half == 0 else 2
                    r1 = etp.tile([128, 1], f32, tag="r1", name="r1", bufs=4)
                    nc.vector.reciprocal(r1[:], attn_ps[hp][:, zc:zc + 1])
                    nc.vector.memset(attnZ[hp], 0.0)
                    nc.vector.tensor_scalar_mul(
                        attnZ[hp][0:64, 0:32],
                        attn_ps[hp][0:64, och:och + 32], r1[0:64, :])
                    nc.vector.tensor_scalar_mul(
                        attnZ[hp][64:128, 32:64],
                        attn_ps[hp][64:128, och + 32:och + 64],
                        r1[64:128, :])

            # ---- stage 2 + proj ----
            with ExitStack() as ph:
                s2sb = ph.enter_context(tc.tile_pool(name="s2sb", bufs=6))
                osb = ph.enter_context(tc.tile_pool(name="osb", bufs=3))
                s2_ps = ph.enter_context(
                    tc.tile_pool(name="s2ps", bufs=2, space="PSUM"))
                g_ps = ph.enter_context(
                    tc.tile_pool(name="gps", bufs=2, space="PSUM"))
                z_ps = ph.enter_context(
                    tc.tile_pool(name="zps", bufs=2, space="PSUM"))
                pr_ps = ph.enter_context(
                    tc.tile_pool(name="prps", bufs=2, space="PSUM"))

                def s2_chunk(nt):
                    for cc in range(2):
                        gp = g_ps.tile([128, 512], f32, tag="g", name="gt")
                        zp = z_ps.tile([128, 512], f32, tag="z", name="zt")
                        for half in range(2):
                            hp = 2 * cc + half
                            sp = s2_ps.tile([128, 512], f32, tag="s2",
                                            name="s2t")
                            nc.tensor.matmul(
                                sp[:],
                                abd_t[:, 256 * cc + 128 * half:
                                      256 * cc + 128 * (half + 1)],
                                q_sb[cc][:, 512 * nt:512 * (nt + 1)],
                                start=True, stop=True)
                            e2 = s2sb.tile([128, 512], bf16, tag="e2",
                                           name="e2")
                            nc.scalar.activation(e2[:], sp[:], AF.Exp,
                                                 scale=SCALE)
                            nc.tensor.matmul(
                                gp[64 * half:64 * half + 64, :],
                                attnZ[hp], e2[:], start=True, stop=True,
                                skip_group_check=True)
                            nc.tensor.matmul(
                                zp[64 * half:64 * half + 64, :],
                                hsel[:], e2[:], start=True, stop=True,
                                skip_group_check=True)
                        rb = s2sb.tile([128, 512], f32, tag="rb", name="rbt")
                        nc.vector.reciprocal_approx_fast(rb[:], zp[:])
                        tsc = s2sb.tile([128, 512], bf16, tag="ts",
                                        name="tsc")
                        nc.vector.tensor_tensor(tsc[:], gp[:], rb[:],
                                                ALU.mult)
                        sl = att_out[cc][:, 512 * nt:512 * (nt + 1)]
                        nc.vector.tensor_tensor(sl, tsc[:], sl, ALU.add)

                # process chunk pairs (b=0, rc), (b=1, rc) then proj rc
                for rc in range(4):
                    s2_chunk(rc)          # b=0 chunk
                    s2_chunk(4 + rc)      # b=1 chunk
                    for mc in range(2):
                        ot = osb.tile([128, 16, 32, 2], bf16, tag="ot",
                                      name="ott")
                        for b in range(2):
                            pp = pr_ps.tile([128, 512], f32, tag="tp",
                                            name="prt")
                            for kc in range(2):
                                nc.tensor.matmul(
                                    pp[:],
                                    pwv[:, kc, 128 * mc:128 * (mc + 1)],
                                    att_out[kc][:, 2048 * b + 512 * rc:
                                                2048 * b + 512 * (rc + 1)],
                                    start=(kc == 0), stop=(kc == 1))
                            ppv = pp[:].rearrange("p (r q one) -> p r q one",
                                                  r=16, q=32, one=1)
                            nc.scalar.add(ot[:, :, :, b:b + 1], ppv,
                                          pb[:, mc:mc + 1])
                        nc.sync.dma_start(
                            OUT[mc, :, 1024 * rc:1024 * (rc + 1)],
                            ot[:].rearrange("p r q two -> p (r q two)"))

    nc.compile()
    return nc


def _prep_consts(qkv_w, qkv_s, qkv_b, pe_w, pe_s, pe_b, proj_w, proj_s,
                 proj_b):
    import ml_dtypes
    f = np.float32
    bf = ml_dtypes.bfloat16
    w = np.asarray(qkv_w, f).copy()          # [768, 256, 3, 3]
    dif = (w[:, :, 0, 1] + w[:, :, 1, 0] + w[:, :, 1, 1] + w[:, :, 1, 2]
           + w[:, :, 2, 1])
    w[:, :, 1, 1] -= THETA * dif
    w *= np.asarray(qkv_s, f)[:, None, None, None]

    # 1-D Winograd F(2,3) weight transform along kx:
    # What[0]=w0, What[1]=(w0+w1+w2)/2, What[2]=(w0-w1+w2)/2, What[3]=w2
    w0, w1, w2 = w[:, :, :, 0], w[:, :, :, 1], w[:, :, :, 2]
    wh = np.stack([w0, (w0 + w1 + w2) * 0.5, (w0 - w1 + w2) * 0.5, w2],
                  axis=0)                     # [4, 768, 256, 3(ky)]
    # WQW[mc, p, j, ky, kc, o'] = wh[j, 128*mc+o', 128*kc+p, ky]
    wh = wh.reshape(4, 6, 128, 2, 128, 3)     # [j, mc, o', kc, p, ky]
    wqw = np.ascontiguousarray(
        wh.transpose(1, 4, 0, 5, 3, 2)).astype(bf)  # [mc,p,j,ky,kc,o']

    bq = np.ascontiguousarray(np.asarray(qkv_b, f).reshape(6, 128).T)

    pe_wf = np.asarray(pe_w, f)[:, 0] * np.asarray(pe_s, f)[:, None, None]
    pew = np.zeros((128, 2, 9), f)
    for kc in range(2):
        for s in range(9):
            pew[:, kc, s] = pe_wf[128 * kc:128 * (kc + 1), s // 3, s % 3]

    pwm = np.asarray(proj_w, f)[:, :, 0, 0] * np.asarray(proj_s, f)[:, None]
    pw = np.ascontiguousarray(
        pwm.T.reshape(2, 128, 256).transpose(1, 0, 2).reshape(
            128, 512)).astype(bf)
    pbv = np.asarray(proj_b, f) + pwm @ np.asarray(pe_b, f)
    pb = np.ascontiguousarray(pbv.reshape(2, 128).T)

    return dict(wqw=wqw, bq=bq, pew=pew, pw=pw, pb=pb)


def kernel(x, qkv_w, qkv_s, qkv_b, pe_w, pe_s, pe_b, proj_w, proj_s, proj_b):
    import ml_dtypes
    from concourse.bass_utils import run_bass_kernel_spmd

    if "nc" not in _cache:
        _cache["nc"] = _build()
    nc = _cache["nc"]

    consts = _prep_consts(qkv_w, qkv_s, qkv_b, pe_w, pe_s, pe_b, proj_w,
                          proj_s, proj_b)
    x = np.asarray(x, np.float32).astype(ml_dtypes.bfloat16)
    in_maps = []
    for b in range(B):
        m = dict(consts)
        m["x"] = np.ascontiguousarray(x[b].reshape(2, 128, HW))
        in_maps.append(m)

    res = run_bass_kernel_spmd(nc, in_maps, list(range(N_CORES)), trace=False)
    out = np.empty((B, C, H, W), np.float32)
    for b in range(B):
        o = np.asarray(res.results[b]["out"], np.float32).reshape(C, HW)
        out[b] = o.reshape(C, H, W)
    return out


# revision 14
# speedup vs baseline: 1.1663x; 1.1663x over previous
"""Trainium2 Bass kernel for agent attention (sparse_attention problem).

Per-core work (data-parallel over batch B=8 across 8 NeuronCores):
  x[b] [256, 64, 64] -> qkv 3x3 conv (dif-conv + BN folded into weights)
  -> agent attention (8 heads, d=32, 64 agent tokens)
  -> depthwise 3x3 pe conv on v -> 1x1 proj.

v5: 1-D Winograd F(2,3) along W for the qkv conv (6 MACs/output instead
of 9): DVE builds 4 transformed input planes U[j] (even/odd column
combinations, padded rows), PE accumulates P[j] = sum_{ky,kc}
What[j,ky,kc]^T U[j] per 16-row chunk into 4 PSUM banks, and a DVE
"step-2" combines P0..P3 into the two output column parities with the
bias folded in (writes q/k/v directly - no separate evacuation).
q/k/v/att_out live in a column-parity-interleaved layout [oc, b, r, q];
all downstream consumers use matching access patterns, and the proj
epilogue re-interleaves pixels on GpSimd before contiguous output DMA.
Depthwise pe conv runs on GpSimd. v^T via batched XBAR DMA transposes.
"""
import numpy as np

NUM_HEADS = 8
AGENT_NUM = 64
THETA = 0.7
C = 256
H = W = 64
HW = H * W
D = C // NUM_HEADS          # 32
N_CORES = 8
B = 8
PS = 8

_cache = {}


def _build():
    import concourse.bass as bass
    import concourse.tile as tile
    from concourse import bacc, mybir

    f32 = mybir.dt.float32
    bf16 = mybir.dt.bfloat16
    AF = mybir.ActivationFunctionType
    ALU = mybir.AluOpType
    AX = mybir.AxisListType

    nc = bacc.Bacc("TRN2", target_bir_lowering=False, debug=False,
                   enable_asserts=True, num_devices=N_CORES)

    X = nc.dram_tensor("x", [2, 128, HW], bf16, kind="ExternalInput").ap()
    # WQW[mc, p, j, ky, kc, o'] = What[j, ky][128*mc+o', 128*kc+p]
    WQW = nc.dram_tensor("wqw", [6, 128, 4, 3, 2, 128], bf16,
                         kind="ExternalInput").ap()
    BQ = nc.dram_tensor("bq", [128, 6], f32, kind="ExternalInput").ap()
    PEW = nc.dram_tensor("pew", [128, 2, 9], f32, kind="ExternalInput").ap()
    PW = nc.dram_tensor("pw", [128, 2 * 256], bf16, kind="ExternalInput").ap()
    PB = nc.dram_tensor("pb", [128, 2], f32, kind="ExternalInput").ap()
    OUT = nc.dram_tensor("out", [2, 128, HW], bf16, kind="ExternalOutput").ap()

    # softmax exp scale: d^-0.5, with the 1/64 agent-pool mean folded in
    SCALE = (D ** -0.5) / (PS * PS)

    with tile.TileContext(nc) as tc:
        from contextlib import ExitStack
        with ExitStack() as top:
            pers = top.enter_context(tc.tile_pool(name="pers", bufs=1))
            xpool_cm = tc.tile_pool(name="xp", bufs=1)
            xpool = xpool_cm.__enter__()
            x_sb = [xpool.tile([128, HW], bf16, tag=f"x{i}", name=f"x{i}")
                    for i in range(2)]
            # shuffled layout [oc, b(2), r(64), q(32)]: pixel (r, 2q+b)
            q_sb = [pers.tile([128, HW], bf16, tag=f"q{i}", name=f"q{i}")
                    for i in range(2)]
            k_sb = [pers.tile([128, HW], bf16, tag=f"k{i}", name=f"k{i}")
                    for i in range(2)]
            v_sb = [pers.tile([128, HW], bf16, tag=f"v{i}", name=f"v{i}")
                    for i in range(2)]
            att_out = [pers.tile([128, HW], bf16, tag=f"ao{i}", name=f"ao{i}")
                       for i in range(2)]
            # U[j][kc]: transformed input planes [128, 66 rows, 32 q]
            u_t = [[pers.tile([128, 66, 32], bf16, tag=f"u{j}{kc}",
                              name=f"u{j}{kc}") for kc in range(2)]
                   for j in range(4)]
            wq_all = pers.tile([128, 6, 4, 3, 2, 128], bf16, tag="wq",
                               name="wq")
            bq = pers.tile([128, 6], f32, tag="bq", name="bq")
            pew = pers.tile([128, 2, 9], f32, tag="pew", name="pew")
            asum_t = pers.tile([128, 128], f32, tag="asum", name="asum")
            a_sum = [asum_t[:, 64 * i:64 * (i + 1)] for i in range(2)]
            abd_t = pers.tile([128, 512], bf16, tag="abd", name="abd")
            abd = [abd_t[:, 256 * i:256 * (i + 1)] for i in range(2)]
            az_t = pers.tile([128, 4 * 64], bf16, tag="az", name="az")
            attnZ = [az_t[:, 64 * i:64 * (i + 1)] for i in range(4)]
            pw = pers.tile([128, 2 * 256], bf16, tag="pw", name="pwt")
            pb = pers.tile([128, 2], f32, tag="pb", name="pbt")
            hsel = pers.tile([128, 64], bf16, tag="hsel", name="hsel")
            vts_t = pers.tile([128, 32, 260], bf16, tag="vts", name="vts")
            pwv = pw[:].rearrange("p (a b) -> p a b", a=2, b=256)

            nc.vector.memset(hsel[:], 0.0)
            nc.vector.memset(hsel[0:64, 0:32], 1.0)
            nc.vector.memset(hsel[64:128, 32:64], 1.0)
            nc.vector.memset(vts_t[:, :, 64:66], 1.0)
            nc.vector.memset(vts_t[:, :, 194:196], 1.0)
            for j in range(4):
                for kc in range(2):
                    nc.vector.memset(u_t[j][kc][:, 0:1, :], 0.0)
                    nc.vector.memset(u_t[j][kc][:, 65:66, :], 0.0)

            # x first (the input transform gates everything), then weights
            for kc in range(2):
                nc.sync.dma_start(x_sb[kc][:, 0:2048], X[kc, :, 0:2048])
                nc.sync.dma_start(x_sb[kc][:, 2048:4096], X[kc, :, 2048:4096])
            for mc in (4, 5, 0, 1, 2, 3):
                nc.sync.dma_start(wq_all[:, mc], WQW[mc])
            nc.sync.dma_start(bq[:], BQ[:])
            nc.sync.dma_start(pew[:], PEW[:])
            nc.sync.dma_start(pw[:], PW[:])
            nc.sync.dma_start(pb[:], PB[:])

            # ---- input transform: U[j][kc][:, 1+r, q] over x rows r ----
            # xe = x[r, 2q], xo = x[r, 2q+1]
            # U0 = xo[q-1] - xo[q]  (q=0: -xo[0])
            # U1 = xe[q] + xo[q]
            # U2 = xo[q] - xe[q]
            # U3 = xe[q] - xe[q+1]  (q=31: xe[31])
            # row-halved so each half only gates on its x half-DMA
            for rh in range(2):
                for j in range(4):
                    for kc in range(2):
                        xv = x_sb[kc][:].rearrange(
                            "p (r q two) -> p r two q", r=64, q=32, two=2)
                        r0, r1 = 32 * rh, 32 * rh + 32
                        xe = xv[:, r0:r1, 0:1, :]
                        xo = xv[:, r0:r1, 1:2, :]
                        uv = u_t[j][kc][:].rearrange(
                            "p r (one q) -> p r one q", one=1,
                            q=32)[:, 1 + r0:1 + r1]
                        if j == 0:
                            nc.vector.tensor_tensor(
                                uv[:, :, :, 1:32], xo[:, :, :, 0:31],
                                xo[:, :, :, 1:32], ALU.subtract)
                            nc.vector.tensor_scalar_mul(
                                uv[:, :, :, 0:1], xo[:, :, :, 0:1], -1.0)
                        elif j == 1:
                            nc.vector.tensor_tensor(
                                uv, xe, xo, ALU.add)
                        elif j == 2:
                            nc.vector.tensor_tensor(
                                uv, xo, xe, ALU.subtract)
                        else:
                            nc.vector.tensor_tensor(
                                uv[:, :, :, 0:31], xe[:, :, :, 0:31],
                                xe[:, :, :, 1:32], ALU.subtract)
                            nc.vector.tensor_copy(
                                uv[:, :, :, 31:32], xe[:, :, :, 31:32])

            xpool_cm.__exit__(None, None, None)

            with ExitStack() as ph:
                cps = ph.enter_context(
                    tc.tile_pool(name="cps", bufs=8, space="PSUM"))
                vtt = ph.enter_context(tc.tile_pool(name="vtt", bufs=2))
                stp = ph.enter_context(tc.tile_pool(name="stp", bufs=4))

                # one conv output group: 128 out-channels (mc), shuffled
                # layout; 4 chunks of 16 rows; P[j] accumulated on PE,
                # combined + biased on DVE, written straight to q/k/v
                def conv_group(mc):
                    if mc < 2:
                        dst_t = q_sb[mc]
                    elif mc < 4:
                        dst_t = k_sb[mc - 2]
                    else:
                        dst_t = v_sb[mc - 4]
                    bias = bq[:, mc:mc + 1]
                    for c in range(4):
                        ps = []
                        for j in range(4):
                            p_t = cps.tile([128, 512], f32, tag="cps",
                                           name="cpst")
                            for i, (ky, kc) in enumerate(
                                    (ky, kc) for ky in range(3)
                                    for kc in range(2)):
                                rhs = u_t[j][kc][:, 16 * c + ky:
                                                 16 * c + ky + 16, :]
                                nc.tensor.matmul(
                                    p_t[:], wq_all[:, mc, j, ky, kc, :],
                                    rhs, start=(i == 0), stop=(i == 5))
                            ps.append(p_t)
                        # step-2: b=0 -> P0+P1+P2+bias ; b=1 -> P1-P2-P3+bias
                        # ACT evacuates the four PSUM banks to SBUF bf16
                        # (folding the bias and the P3 sign), then all-SBUF
                        # TT chains run on DVE (b=0) and GpSimd (b=1)
                        p0b = stp.tile([128, 512], bf16, tag="p0b",
                                       name="p0b", bufs=6)
                        nc.scalar.add(p0b[:], ps[0][:], bias)
                        p1e = stp.tile([128, 512], bf16, tag="p1e",
                                       name="p1e", bufs=6)
                        nc.scalar.copy(p1e[:], ps[1][:])
                        p2e = stp.tile([128, 512], bf16, tag="p2e",
                                       name="p2e", bufs=6)
                        nc.scalar.copy(p2e[:], ps[2][:])
                        p3n = stp.tile([128, 512], bf16, tag="p3n",
                                       name="p3n", bufs=6)
                        nc.scalar.activation(p3n[:], ps[3][:], AF.Identity,
                                             bias=bias, scale=-1.0)
                        t0b = stp.tile([128, 512], bf16, tag="t0b",
                                       name="t0b", bufs=2)
                        nc.vector.tensor_tensor(t0b[:], p0b[:], p1e[:],
                                                ALU.add)
                        nc.vector.tensor_tensor(
                            dst_t[:, 512 * c:512 * (c + 1)], t0b[:],
                            p2e[:], ALU.add)
                        t1b = stp.tile([128, 512], bf16, tag="t1b",
                                       name="t1b", bufs=2)
                        nc.gpsimd.tensor_tensor(t1b[:], p1e[:], p2e[:],
                                                ALU.subtract)
                        nc.gpsimd.tensor_tensor(
                            dst_t[:, 2048 + 512 * c:2048 + 512 * (c + 1)],
                            t1b[:], p3n[:], ALU.add)

                # depthwise pe conv on GpSimd in the shuffled layout,
                # accumulating into att_out; center tap first (overwrite)
                def pe_conv(cc, g):
                    vv = v_sb[cc][:].rearrange("p (b r q) -> p b r q",
                                               b=2, r=64, q=32)
                    ao = att_out[cc][:].rearrange("p (b r q) -> p b r q",
                                                  b=2, r=64, q=32)
                    g0, g1 = 32 * g, 32 * g + 32
                    for b in range(2):
                        nc.vector.tensor_scalar_mul(
                            ao[:, b:b + 1, g0:g1, :],
                            vv[:, b:b + 1, g0:g1, :], pew[:, cc, 4:5])
                    for s in (0, 1, 2, 3, 5, 6, 7, 8):
                        dy, dx = s // 3 - 1, s % 3 - 1
                        r_lo = max(g0, -dy)
                        r_hi = min(g1, 64 - dy)
                        for b in range(2):
                            if dx == 0:
                                sb, q_lo, q_hi, sq = b, 0, 32, 0
                            elif dx == 1:
                                # src col 2q+b+1
                                if b == 0:
                                    sb, q_lo, q_hi, sq = 1, 0, 32, 0
                                else:
                                    sb, q_lo, q_hi, sq = 0, 0, 31, 1
                            else:
                                # src col 2q+b-1
                                if b == 1:
                                    sb, q_lo, q_hi, sq = 0, 0, 32, 0
                                else:
                                    sb, q_lo, q_hi, sq = 1, 1, 32, -1
                            dst = ao[:, b:b + 1, r_lo:r_hi, q_lo:q_hi]
                            src = vv[:, sb:sb + 1, r_lo + dy:r_hi + dy,
                                     q_lo + sq:q_hi + sq]
                            nc.vector.scalar_tensor_tensor(
                                dst, src, pew[:, cc, s:s + 1], dst,
                                ALU.mult, ALU.add)

                # v first
                conv_group(4)
                conv_group(5)

                # v^T via 4 batched XBAR DMA transposes + copies into the
                # interleaved [64ch | ones | 64ch] layout (on ACT engine)
                for cc in range(2):
                    for half in range(2):
                        vt_stg = vtt.tile([128, 32, 64], bf16, tag="vstg",
                                          name="vstg")
                        nc.sync.dma_start_transpose(
                            vt_stg[:],
                            v_sb[cc][64 * half:64 * (half + 1), :])
                        nc.scalar.copy(
                            vts_t[:, :, 130 * cc + 66 * half:
                                  130 * cc + 66 * half + 64],
                            vt_stg[:])

                def pooling(cc):
                    qsrc = q_sb[cc]
                    tmp = stp.tile([128, 2, 8, 8, 8], f32, tag="pool",
                                   name="pool", bufs=1)
                    for b in range(2):
                        qv = qsrc[:, 2048 * b:2048 * (b + 1)].rearrange(
                            "p (by rr bx qq) -> p by rr bx qq",
                            by=8, rr=8, bx=8, qq=4)
                        nc.vector.tensor_reduce(
                            tmp[:, b], qv, AX.X, ALU.add)
                    tv = tmp[:].rearrange("p b by rr bx -> p by bx b rr")
                    nc.vector.tensor_reduce(a_sum[cc], tv, AX.XY, ALU.add)

                def abd_build():
                    nc.vector.memset(abd_t[:], 0.0)
                    for cc in range(2):
                        for j in range(4):
                            nc.vector.tensor_copy(
                                abd[cc][32 * j:32 * j + 32,
                                        64 * j:64 * j + 64],
                                a_sum[cc][32 * j:32 * j + 32, :])

                pe_conv(0, 0)
                conv_group(0)
                pe_conv(0, 1)
                conv_group(1)
                pooling(0)
                pooling(1)
                abd_build()
                pe_conv(1, 0)
                conv_group(2)
                pe_conv(1, 1)
                conv_group(3)

            # ---- stage 1 ----
            with ExitStack() as ph:
                st_ps = ph.enter_context(
                    tc.tile_pool(name="stps", bufs=4, space="PSUM"))
                at_ps = ph.enter_context(
                    tc.tile_pool(name="atps", bufs=4, space="PSUM"))
                etp = ph.enter_context(tc.tile_pool(name="etp", bufs=1))
                attn_ps = [at_ps.tile([128, 66], f32, tag="at", name="at")
                           for _ in range(4)]
                for chp in range(16):   # px-chunk pairs
                    for cc in range(2):
                        sp = st_ps.tile([128, 512], f32, tag="st",
                                        name="stt")
                        for u in range(2):
                            ch = 2 * chp + u
                            nc.tensor.matmul(
                                sp[:, 256 * u:256 * (u + 1)],
                                k_sb[cc][:, 128 * ch:128 * (ch + 1)],
                                abd[cc], start=True, stop=True,
                                skip_group_check=True)
                        et = etp.tile([128, 512], bf16, tag="et",
                                      name="et", bufs=6)
                        nc.scalar.activation(et[:], sp[:], AF.Exp,
                                             scale=SCALE)
                        for u in range(2):
                            ch = 2 * chp + u
                            for half in range(2):
                                hp = 2 * cc + half
                                rhs = vts_t[:, ch, 130 * cc + 64 * half:
                                            130 * cc + 64 * half + 66]
                                nc.tensor.matmul(
                                    attn_ps[hp][:],
                                    et[:, 256 * u + 128 * half:
                                       256 * u + 128 * (half + 1)],
                                    rhs, start=(ch == 0), stop=(ch == 31))

                for hp in range(4):
                    half = hp % 2
                    zc = 64 if half == 0 else 0
                    och = 0 if half == 0 else 2
                    r1 = etp.tile([128, 1], f32, tag="r1", name="r1", bufs=4)
                    nc.vector.reciprocal(r1[:], attn_ps[hp][:, zc:zc + 1])
                    nc.vector.memset(attnZ[hp], 0.0)
                    nc.vector.tensor_scalar_mul(
                        attnZ[hp][0:64, 0:32],
                        attn_ps[hp][0:64, och:och + 32], r1[0:64, :])
                    nc.vector.tensor_scalar_mul(
                        attnZ[hp][64:128, 32:64],
                        attn_ps[hp][64:128, och + 32:och + 64],
                        r1[64:128, :])

            # ---- stage 2 + proj ----
            with ExitStack() as ph:
                s2sb = ph.enter_context(tc.tile_pool(name="s2sb", bufs=6))
                osb = ph.enter_context(tc.tile_pool(name="osb", bufs=3))
                s2_ps = ph.enter_context(
                    tc.tile_pool(name="s2ps", bufs=2, space="PSUM"))
                g_ps = ph.enter_context(
                    tc.tile_pool(name="gps", bufs=2, space="PSUM"))
                z_ps = ph.enter_context(
                    tc.tile_pool(name="zps", bufs=2, space="PSUM"))
                pr_ps = ph.enter_context(
                    tc.tile_pool(name="prps", bufs=2, space="PSUM"))

                def s2_chunk(nt):
                    for cc in range(2):
                        gp = g_ps.tile([128, 512], f32, tag="g", name="gt")
                        zp = z_ps.tile([128, 512], f32, tag="z", name="zt")
                        for half in range(2):
                            hp = 2 * cc + half
                            sp = s2_ps.tile([128, 512], f32, tag="s2",
                                            name="s2t")
                            nc.tensor.matmul(
                                sp[:],
                                abd_t[:, 256 * cc + 128 * half:
                                      256 * cc + 128 * (half + 1)],
                                q_sb[cc][:, 512 * nt:512 * (nt + 1)],
                                start=True, stop=True)
                            e2 = s2sb.tile([128, 512], bf16, tag="e2",
                                           name="e2")
                            nc.scalar.activation(e2[:], sp[:], AF.Exp,
                                                 scale=SCALE)
                            nc.tensor.matmul(
                                gp[64 * half:64 * half + 64, :],
                                attnZ[hp], e2[:], start=True, stop=True,
                                skip_group_check=True)
                            nc.tensor.matmul(
                                zp[64 * half:64 * half + 64, :],
                                hsel[:], e2[:], start=True, stop=True,
                                skip_group_check=True)
                        rb = s2sb.tile([128, 512], f32, tag="rb", name="rbt")
                        nc.vector.reciprocal_approx_fast(rb[:], zp[:])
                        tsc = s2sb.tile([128, 512], bf16, tag="ts",
                                        name="tsc")
                        nc.vector.tensor_tensor(tsc[:], gp[:], rb[:],
                                                ALU.mult)
                        sl = att_out[cc][:, 512 * nt:512 * (nt + 1)]
                        nc.vector.tensor_tensor(sl, tsc[:], sl, ALU.add)

                # process chunk pairs (b=0, rc), (b=1, rc) then proj rc
                for rc in range(4):
                    s2_chunk(rc)          # b=0 chunk
                    s2_chunk(4 + rc)      # b=1 chunk
                    for mc in range(2):
                        ot = osb.tile([128, 16, 32, 2], bf16, tag="ot",
                                      name="ott")
                        for b in range(2):
                            pp = pr_ps.tile([128, 512], f32, tag="tp",
                                            name="prt")
                            for kc in range(2):
                                nc.tensor.matmul(
                                    pp[:],
                                    pwv[:, kc, 128 * mc:128 * (mc + 1)],
                                    att_out[kc][:, 2048 * b + 512 * rc:
                                                2048 * b + 512 * (rc + 1)],
                                    start=(kc == 0), stop=(kc == 1))
                            ppv = pp[:].rearrange("p (r q one) -> p r q one",
                                                  r=16, q=32, one=1)
                            nc.scalar.add(ot[:, :, :, b:b + 1], ppv,
                                          pb[:, mc:mc + 1])
                        nc.sync.dma_start(
                            OUT[mc, :, 1024 * rc:1024 * (rc + 1)],
                            ot[:].rearrange("p r q two -> p (r q two)"))

    nc.compile()
    return nc


def _prep_consts(qkv_w, qkv_s, qkv_b, pe_w, pe_s, pe_b, proj_w, proj_s,
                 proj_b):
    import ml_dtypes
    f = np.float32
    bf = ml_dtypes.bfloat16
    w = np.asarray(qkv_w, f).copy()          # [768, 256, 3, 3]
    dif = (w[:, :, 0, 1] + w[:, :, 1, 0] + w[:, :, 1, 1] + w[:, :, 1, 2]
           + w[:, :, 2, 1])
    w[:, :, 1, 1] -= THETA * dif
    w *= np.asarray(qkv_s, f)[:, None, None, None]

    # 1-D Winograd F(2,3) weight transform along kx:
    # What[0]=w0, What[1]=(w0+w1+w2)/2, What[2]=(w0-w1+w2)/2, What[3]=w2
    w0, w1, w2 = w[:, :, :, 0], w[:, :, :, 1], w[:, :, :, 2]
    wh = np.stack([w0, (w0 + w1 + w2) * 0.5, (w0 - w1 + w2) * 0.5, w2],
                  axis=0)                     # [4, 768, 256, 3(ky)]
    # WQW[mc, p, j, ky, kc, o'] = wh[j, 128*mc+o', 128*kc+p, ky]
    wh = wh.reshape(4, 6, 128, 2, 128, 3)     # [j, mc, o', kc, p, ky]
    wqw = np.ascontiguousarray(
        wh.transpose(1, 4, 0, 5, 3, 2)).astype(bf)  # [mc,p,j,ky,kc,o']

    bq = np.ascontiguousarray(np.asarray(qkv_b, f).reshape(6, 128).T)

    pe_wf = np.asarray(pe_w, f)[:, 0] * np.asarray(pe_s, f)[:, None, None]
    pew = np.zeros((128, 2, 9), f)
    for kc in range(2):
        for s in range(9):
            pew[:, kc, s] = pe_wf[128 * kc:128 * (kc + 1), s // 3, s % 3]

    pwm = np.asarray(proj_w, f)[:, :, 0, 0] * np.asarray(proj_s, f)[:, None]
    pw = np.ascontiguousarray(
        pwm.T.reshape(2, 128, 256).transpose(1, 0, 2).reshape(
            128, 512)).astype(bf)
    pbv = np.asarray(proj_b, f) + pwm @ np.asarray(pe_b, f)
    pb = np.ascontiguousarray(pbv.reshape(2, 128).T)

    return dict(wqw=wqw, bq=bq, pew=pew, pw=pw, pb=pb)


def kernel(x, qkv_w, qkv_s, qkv_b, pe_w, pe_s, pe_b, proj_w, proj_s, proj_b):
    import ml_dtypes
    from concourse.bass_utils import run_bass_kernel_spmd

    if "nc" not in _cache:
        _cache["nc"] = _build()
    nc = _cache["nc"]

    consts = _prep_consts(qkv_w, qkv_s, qkv_b, pe_w, pe_s, pe_b, proj_w,
                          proj_s, proj_b)
    x = np.asarray(x, np.float32).astype(ml_dtypes.bfloat16)
    in_maps = []
    for b in range(B):
        m = dict(consts)
        m["x"] = np.ascontiguousarray(x[b].reshape(2, 128, HW))
        in_maps.append(m)

    res = run_bass_kernel_spmd(nc, in_maps, list(range(N_CORES)), trace=False)
    out = np.empty((B, C, H, W), np.float32)
    for b in range(B):
        o = np.asarray(res.results[b]["out"], np.float32).reshape(C, HW)
        out[b] = o.reshape(C, H, W)
    return out


# revision 15
# speedup vs baseline: 1.1686x; 1.0019x over previous
"""Trainium2 Bass kernel for agent attention (sparse_attention problem).

Per-core work (data-parallel over batch B=8 across 8 NeuronCores):
  x[b] [256, 64, 64] -> qkv 3x3 conv (dif-conv + BN folded into weights)
  -> agent attention (8 heads, d=32, 64 agent tokens)
  -> depthwise 3x3 pe conv on v -> 1x1 proj.

v5: 1-D Winograd F(2,3) along W for the qkv conv (6 MACs/output instead
of 9): DVE builds 4 transformed input planes U[j] (even/odd column
combinations, padded rows), PE accumulates P[j] = sum_{ky,kc}
What[j,ky,kc]^T U[j] per 16-row chunk into 4 PSUM banks, and a DVE
"step-2" combines P0..P3 into the two output column parities with the
bias folded in (writes q/k/v directly - no separate evacuation).
q/k/v/att_out live in a column-parity-interleaved layout [oc, b, r, q];
all downstream consumers use matching access patterns, and the proj
epilogue re-interleaves pixels on GpSimd before contiguous output DMA.
Depthwise pe conv runs on GpSimd. v^T via batched XBAR DMA transposes.
"""
import numpy as np

NUM_HEADS = 8
AGENT_NUM = 64
THETA = 0.7
C = 256
H = W = 64
HW = H * W
D = C // NUM_HEADS          # 32
N_CORES = 8
B = 8
PS = 8

_cache = {}


def _build():
    import concourse.bass as bass
    import concourse.tile as tile
    from concourse import bacc, mybir

    f32 = mybir.dt.float32
    bf16 = mybir.dt.bfloat16
    AF = mybir.ActivationFunctionType
    ALU = mybir.AluOpType
    AX = mybir.AxisListType

    nc = bacc.Bacc("TRN2", target_bir_lowering=False, debug=False,
                   enable_asserts=True, num_devices=N_CORES)

    X = nc.dram_tensor("x", [2, 128, HW], bf16, kind="ExternalInput").ap()
    # WQW[mc, p, j, ky, kc, o'] = What[j, ky][128*mc+o', 128*kc+p]
    WQW = nc.dram_tensor("wqw", [6, 128, 4, 3, 2, 128], bf16,
                         kind="ExternalInput").ap()
    BQ = nc.dram_tensor("bq", [128, 6], f32, kind="ExternalInput").ap()
    PEW = nc.dram_tensor("pew", [128, 2, 9], f32, kind="ExternalInput").ap()
    PW = nc.dram_tensor("pw", [128, 2 * 256], bf16, kind="ExternalInput").ap()
    PB = nc.dram_tensor("pb", [128, 2], f32, kind="ExternalInput").ap()
    OUT = nc.dram_tensor("out", [2, 128, HW], bf16, kind="ExternalOutput").ap()

    # softmax exp scale: d^-0.5, with the 1/64 agent-pool mean folded in
    SCALE = (D ** -0.5) / (PS * PS)

    with tile.TileContext(nc) as tc:
        from contextlib import ExitStack
        with ExitStack() as top:
            pers = top.enter_context(tc.tile_pool(name="pers", bufs=1))
            xpool_cm = tc.tile_pool(name="xp", bufs=1)
            xpool = xpool_cm.__enter__()
            x_sb = [xpool.tile([128, HW], bf16, tag=f"x{i}", name=f"x{i}")
                    for i in range(2)]
            # shuffled layout [oc, b(2), r(64), q(32)]: pixel (r, 2q+b)
            q_sb = [pers.tile([128, HW], bf16, tag=f"q{i}", name=f"q{i}")
                    for i in range(2)]
            k_sb = [pers.tile([128, HW], bf16, tag=f"k{i}", name=f"k{i}")
                    for i in range(2)]
            v_sb = [pers.tile([128, HW], bf16, tag=f"v{i}", name=f"v{i}")
                    for i in range(2)]
            att_out = [pers.tile([128, HW], bf16, tag=f"ao{i}", name=f"ao{i}")
                       for i in range(2)]
            # U[j][kc]: transformed input planes [128, 66 rows, 32 q]
            u_t = [[pers.tile([128, 66, 32], bf16, tag=f"u{j}{kc}",
                              name=f"u{j}{kc}") for kc in range(2)]
                   for j in range(4)]
            wq_all = pers.tile([128, 6, 4, 3, 2, 128], bf16, tag="wq",
                               name="wq")
            bq = pers.tile([128, 6], f32, tag="bq", name="bq")
            pew = pers.tile([128, 2, 9], f32, tag="pew", name="pew")
            asum_t = pers.tile([128, 128], f32, tag="asum", name="asum")
            a_sum = [asum_t[:, 64 * i:64 * (i + 1)] for i in range(2)]
            abd_t = pers.tile([128, 512], bf16, tag="abd", name="abd")
            abd = [abd_t[:, 256 * i:256 * (i + 1)] for i in range(2)]
            az_t = pers.tile([128, 4 * 64], bf16, tag="az", name="az")
            attnZ = [az_t[:, 64 * i:64 * (i + 1)] for i in range(4)]
            pw = pers.tile([128, 2 * 256], bf16, tag="pw", name="pwt")
            pb = pers.tile([128, 2], f32, tag="pb", name="pbt")
            hsel = pers.tile([128, 64], bf16, tag="hsel", name="hsel")
            vts_t = pers.tile([128, 32, 260], bf16, tag="vts", name="vts")
            pwv = pw[:].rearrange("p (a b) -> p a b", a=2, b=256)

            nc.vector.memset(hsel[:], 0.0)
            nc.vector.memset(hsel[0:64, 0:32], 1.0)
            nc.vector.memset(hsel[64:128, 32:64], 1.0)
            nc.vector.memset(vts_t[:, :, 64:66], 1.0)
            nc.vector.memset(vts_t[:, :, 194:196], 1.0)
            for j in range(4):
                for kc in range(2):
                    nc.vector.memset(u_t[j][kc][:, 0:1, :], 0.0)
                    nc.vector.memset(u_t[j][kc][:, 65:66, :], 0.0)

            # x first (the input transform gates everything), then weights
            for kc in range(2):
                nc.sync.dma_start(x_sb[kc][:, 0:2048], X[kc, :, 0:2048])
                nc.sync.dma_start(x_sb[kc][:, 2048:4096], X[kc, :, 2048:4096])
            for mc in (4, 5):
                nc.sync.dma_start(wq_all[:, mc], WQW[mc])
            nc.sync.dma_start(bq[:], BQ[:])
            nc.sync.dma_start(pew[:], PEW[:])
            nc.sync.dma_start(pb[:], PB[:])

            # ---- input transform: U[j][kc][:, 1+r, q] over x rows r ----
            # xe = x[r, 2q], xo = x[r, 2q+1]
            # U0 = xo[q-1] - xo[q]  (q=0: -xo[0])
            # U1 = xe[q] + xo[q]
            # U2 = xo[q] - xe[q]
            # U3 = xe[q] - xe[q+1]  (q=31: xe[31])
            # row-halved so each half only gates on its x half-DMA
            for rh in range(2):
                for j in range(4):
                    for kc in range(2):
                        xv = x_sb[kc][:].rearrange(
                            "p (r q two) -> p r two q", r=64, q=32, two=2)
                        r0, r1 = 32 * rh, 32 * rh + 32
                        xe = xv[:, r0:r1, 0:1, :]
                        xo = xv[:, r0:r1, 1:2, :]
                        uv = u_t[j][kc][:].rearrange(
                            "p r (one q) -> p r one q", one=1,
                            q=32)[:, 1 + r0:1 + r1]
                        if j == 0:
                            nc.vector.tensor_tensor(
                                uv[:, :, :, 1:32], xo[:, :, :, 0:31],
                                xo[:, :, :, 1:32], ALU.subtract)
                            nc.vector.tensor_scalar_mul(
                                uv[:, :, :, 0:1], xo[:, :, :, 0:1], -1.0)
                        elif j == 1:
                            nc.vector.tensor_tensor(
                                uv, xe, xo, ALU.add)
                        elif j == 2:
                            nc.vector.tensor_tensor(
                                uv, xo, xe, ALU.subtract)
                        else:
                            nc.vector.tensor_tensor(
                                uv[:, :, :, 0:31], xe[:, :, :, 0:31],
                                xe[:, :, :, 1:32], ALU.subtract)
                            nc.vector.tensor_copy(
                                uv[:, :, :, 31:32], xe[:, :, :, 31:32])

            xpool_cm.__exit__(None, None, None)

            with ExitStack() as ph:
                cps = ph.enter_context(
                    tc.tile_pool(name="cps", bufs=8, space="PSUM"))
                vtt = ph.enter_context(tc.tile_pool(name="vtt", bufs=2))
                stp = ph.enter_context(tc.tile_pool(name="stp", bufs=4))

                # one conv output group: 128 out-channels (mc), shuffled
                # layout; 4 chunks of 16 rows; P[j] accumulated on PE,
                # combined + biased on DVE, written straight to q/k/v
                def conv_group(mc):
                    if mc < 2:
                        dst_t = q_sb[mc]
                    elif mc < 4:
                        dst_t = k_sb[mc - 2]
                    else:
                        dst_t = v_sb[mc - 4]
                    bias = bq[:, mc:mc + 1]
                    for c in range(4):
                        ps = []
                        for j in range(4):
                            p_t = cps.tile([128, 512], f32, tag="cps",
                                           name="cpst")
                            for i, (ky, kc) in enumerate(
                                    (ky, kc) for ky in range(3)
                                    for kc in range(2)):
                                rhs = u_t[j][kc][:, 16 * c + ky:
                                                 16 * c + ky + 16, :]
                                nc.tensor.matmul(
                                    p_t[:], wq_all[:, mc, j, ky, kc, :],
                                    rhs, start=(i == 0), stop=(i == 5))
                            ps.append(p_t)
                        # step-2: b=0 -> P0+P1+P2+bias ; b=1 -> P1-P2-P3+bias
                        # ACT evacuates the four PSUM banks to SBUF bf16
                        # (folding the bias and the P3 sign), then all-SBUF
                        # TT chains run on DVE (b=0) and GpSimd (b=1)
                        p0b = stp.tile([128, 512], bf16, tag="p0b",
                                       name="p0b", bufs=6)
                        nc.scalar.add(p0b[:], ps[0][:], bias)
                        p1e = stp.tile([128, 512], bf16, tag="p1e",
                                       name="p1e", bufs=6)
                        nc.scalar.copy(p1e[:], ps[1][:])
                        p2e = stp.tile([128, 512], bf16, tag="p2e",
                                       name="p2e", bufs=6)
                        nc.scalar.copy(p2e[:], ps[2][:])
                        p3n = stp.tile([128, 512], bf16, tag="p3n",
                                       name="p3n", bufs=6)
                        nc.scalar.activation(p3n[:], ps[3][:], AF.Identity,
                                             bias=bias, scale=-1.0)
                        t0b = stp.tile([128, 512], bf16, tag="t0b",
                                       name="t0b", bufs=2)
                        nc.vector.tensor_tensor(t0b[:], p0b[:], p1e[:],
                                                ALU.add)
                        nc.vector.tensor_tensor(
                            dst_t[:, 512 * c:512 * (c + 1)], t0b[:],
                            p2e[:], ALU.add)
                        t1b = stp.tile([128, 512], bf16, tag="t1b",
                                       name="t1b", bufs=2)
                        nc.gpsimd.tensor_tensor(t1b[:], p1e[:], p2e[:],
                                                ALU.subtract)
                        nc.gpsimd.tensor_tensor(
                            dst_t[:, 2048 + 512 * c:2048 + 512 * (c + 1)],
                            t1b[:], p3n[:], ALU.add)

                # depthwise pe conv on GpSimd in the shuffled layout,
                # accumulating into att_out; center tap first (overwrite)
                def pe_conv(cc, g):
                    vv = v_sb[cc][:].rearrange("p (b r q) -> p b r q",
                                               b=2, r=64, q=32)
                    ao = att_out[cc][:].rearrange("p (b r q) -> p b r q",
                                                  b=2, r=64, q=32)
                    g0, g1 = 32 * g, 32 * g + 32
                    for b in range(2):
                        nc.vector.tensor_scalar_mul(
                            ao[:, b:b + 1, g0:g1, :],
                            vv[:, b:b + 1, g0:g1, :], pew[:, cc, 4:5])
                    for s in (0, 1, 2, 3, 5, 6, 7, 8):
                        dy, dx = s // 3 - 1, s % 3 - 1
                        r_lo = max(g0, -dy)
                        r_hi = min(g1, 64 - dy)
                        for b in range(2):
                            if dx == 0:
                                sb, q_lo, q_hi, sq = b, 0, 32, 0
                            elif dx == 1:
                                # src col 2q+b+1
                                if b == 0:
                                    sb, q_lo, q_hi, sq = 1, 0, 32, 0
                                else:
                                    sb, q_lo, q_hi, sq = 0, 0, 31, 1
                            else:
                                # src col 2q+b-1
                                if b == 1:
                                    sb, q_lo, q_hi, sq = 0, 0, 32, 0
                                else:
                                    sb, q_lo, q_hi, sq = 1, 1, 32, -1
                            dst = ao[:, b:b + 1, r_lo:r_hi, q_lo:q_hi]
                            src = vv[:, sb:sb + 1, r_lo + dy:r_hi + dy,
                                     q_lo + sq:q_hi + sq]
                            nc.vector.scalar_tensor_tensor(
                                dst, src, pew[:, cc, s:s + 1], dst,
                                ALU.mult, ALU.add)

                # v first
                conv_group(4)
                # bulk weight DMAs issued from ACT here: their issue slot
                # sits behind conv(4)'s first evacuations in the ACT queue,
                # so they don't compete with the x DMAs for HBM bandwidth
                for mc in (0, 1, 2, 3):
                    nc.scalar.dma_start(wq_all[:, mc], WQW[mc])
                nc.scalar.dma_start(pw[:], PW[:])
                conv_group(5)

                # v^T via 4 batched XBAR DMA transposes + copies into the
                # interleaved [64ch | ones | 64ch] layout (on ACT engine)
                for cc in range(2):
                    for half in range(2):
                        vt_stg = vtt.tile([128, 32, 64], bf16, tag="vstg",
                                          name="vstg")
                        nc.sync.dma_start_transpose(
                            vt_stg[:],
                            v_sb[cc][64 * half:64 * (half + 1), :])
                        nc.sync.dma_start(
                            vts_t[:, :, 130 * cc + 66 * half:
                                  130 * cc + 66 * half + 64],
                            vt_stg[:])

                def pooling(cc):
                    qsrc = q_sb[cc]
                    tmp = stp.tile([128, 2, 8, 8, 8], f32, tag="pool",
                                   name="pool", bufs=1)
                    for b in range(2):
                        qv = qsrc[:, 2048 * b:2048 * (b + 1)].rearrange(
                            "p (by rr bx qq) -> p by rr bx qq",
                            by=8, rr=8, bx=8, qq=4)
                        nc.vector.tensor_reduce(
                            tmp[:, b], qv, AX.X, ALU.add)
                    tv = tmp[:].rearrange("p b by rr bx -> p by bx b rr")
                    nc.vector.tensor_reduce(a_sum[cc], tv, AX.XY, ALU.add)

                def abd_build():
                    nc.vector.memset(abd_t[:], 0.0)
                    for cc in range(2):
                        for j in range(4):
                            nc.vector.tensor_copy(
                                abd[cc][32 * j:32 * j + 32,
                                        64 * j:64 * j + 64],
                                a_sum[cc][32 * j:32 * j + 32, :])

                pe_conv(0, 0)
                conv_group(0)
                pe_conv(0, 1)
                conv_group(1)
                pooling(0)
                pooling(1)
                abd_build()
                pe_conv(1, 0)
                conv_group(2)
                conv_group(3)
                pe_conv(1, 1)

            # ---- stage 1 ----
            with ExitStack() as ph:
                st_ps = ph.enter_context(
                    tc.tile_pool(name="stps", bufs=4, space="PSUM"))
                at_ps = ph.enter_context(
                    tc.tile_pool(name="atps", bufs=4, space="PSUM"))
                etp = ph.enter_context(tc.tile_pool(name="etp", bufs=1))
                attn_ps = [at_ps.tile([128, 66], f32, tag="at", name="at")
                           for _ in range(4)]
                for chp in range(16):   # px-chunk pairs
                    for cc in range(2):
                        sp = st_ps.tile([128, 512], f32, tag="st",
                                        name="stt")
                        for u in range(2):
                            ch = 2 * chp + u
                            nc.tensor.matmul(
                                sp[:, 256 * u:256 * (u + 1)],
                                k_sb[cc][:, 128 * ch:128 * (ch + 1)],
                                abd[cc], start=True, stop=True,
                                skip_group_check=True)
                        et = etp.tile([128, 512], bf16, tag="et",
                                      name="et", bufs=6)
                        nc.scalar.activation(et[:], sp[:], AF.Exp,
                                             scale=SCALE)
                        for u in range(2):
                            ch = 2 * chp + u
                            for half in range(2):
                                hp = 2 * cc + half
                                rhs = vts_t[:, ch, 130 * cc + 64 * half:
                                            130 * cc + 64 * half + 66]
                                nc.tensor.matmul(
                                    attn_ps[hp][:],
                                    et[:, 256 * u + 128 * half:
                                       256 * u + 128 * (half + 1)],
                                    rhs, start=(ch == 0), stop=(ch == 31))

                for hp in range(4):
                    half = hp % 2
                    zc = 64 if half == 0 else 0
                    och = 0 if half == 0 else 2
                    r1 = etp.tile([128, 1], f32, tag="r1", name="r1", bufs=4)
                    nc.vector.reciprocal(r1[:], attn_ps[hp][:, zc:zc + 1])
                    nc.vector.memset(attnZ[hp], 0.0)
                    nc.vector.tensor_scalar_mul(
                        attnZ[hp][0:64, 0:32],
                        attn_ps[hp][0:64, och:och + 32], r1[0:64, :])
                    nc.vector.tensor_scalar_mul(
                        attnZ[hp][64:128, 32:64],
                        attn_ps[hp][64:128, och + 32:och + 64],
                        r1[64:128, :])

            # ---- stage 2 + proj ----
            with ExitStack() as ph:
                s2sb = ph.enter_context(tc.tile_pool(name="s2sb", bufs=6))
                osb = ph.enter_context(tc.tile_pool(name="osb", bufs=3))
                s2_ps = ph.enter_context(
                    tc.tile_pool(name="s2ps", bufs=2, space="PSUM"))
                g_ps = ph.enter_context(
                    tc.tile_pool(name="gps", bufs=2, space="PSUM"))
                z_ps = ph.enter_context(
                    tc.tile_pool(name="zps", bufs=2, space="PSUM"))
                pr_ps = ph.enter_context(
                    tc.tile_pool(name="prps", bufs=2, space="PSUM"))

                def s2_chunk(nt):
                    for cc in range(2):
                        gp = g_ps.tile([128, 512], f32, tag="g", name="gt")
                        zp = z_ps.tile([128, 512], f32, tag="z", name="zt")
                        for half in range(2):
                            hp = 2 * cc + half
                            sp = s2_ps.tile([128, 512], f32, tag="s2",
                                            name="s2t")
                            nc.tensor.matmul(
                                sp[:],
                                abd_t[:, 256 * cc + 128 * half:
                                      256 * cc + 128 * (half + 1)],
                                q_sb[cc][:, 512 * nt:512 * (nt + 1)],
                                start=True, stop=True)
                            e2 = s2sb.tile([128, 512], bf16, tag="e2",
                                           name="e2")
                            nc.scalar.activation(e2[:], sp[:], AF.Exp,
                                                 scale=SCALE)
                            nc.tensor.matmul(
                                gp[64 * half:64 * half + 64, :],
                                attnZ[hp], e2[:], start=True, stop=True,
                                skip_group_check=True)
                            nc.tensor.matmul(
                                zp[64 * half:64 * half + 64, :],
                                hsel[:], e2[:], start=True, stop=True,
                                skip_group_check=True)
                        rb = s2sb.tile([128, 512], f32, tag="rb", name="rbt")
                        nc.vector.reciprocal_approx_fast(rb[:], zp[:])
                        tsc = s2sb.tile([128, 512], bf16, tag="ts",
                                        name="tsc")
                        nc.vector.tensor_tensor(tsc[:], gp[:], rb[:],
                                                ALU.mult)
                        sl = att_out[cc][:, 512 * nt:512 * (nt + 1)]
                        nc.vector.tensor_tensor(sl, tsc[:], sl, ALU.add)

                # process chunk pairs (b=0, rc), (b=1, rc) then proj rc
                for rc in range(4):
                    s2_chunk(rc)          # b=0 chunk
                    s2_chunk(4 + rc)      # b=1 chunk
                    for mc in range(2):
                        ot = osb.tile([128, 16, 32, 2], bf16, tag="ot",
                                      name="ott")
                        for b in range(2):
                            pp = pr_ps.tile([128, 512], f32, tag="tp",
                                            name="prt")
                            for kc in range(2):
                                nc.tensor.matmul(
                                    pp[:],
                                    pwv[:, kc, 128 * mc:128 * (mc + 1)],
                                    att_out[kc][:, 2048 * b + 512 * rc:
                                                2048 * b + 512 * (rc + 1)],
                                    start=(kc == 0), stop=(kc == 1))
                            ppv = pp[:].rearrange("p (r q one) -> p r q one",
                                                  r=16, q=32, one=1)
                            nc.scalar.add(ot[:, :, :, b:b + 1], ppv,
                                          pb[:, mc:mc + 1])
                        nc.sync.dma_start(
                            OUT[mc, :, 1024 * rc:1024 * (rc + 1)],
                            ot[:].rearrange("p r q two -> p (r q two)"))

    nc.compile()
    return nc


def _prep_consts(qkv_w, qkv_s, qkv_b, pe_w, pe_s, pe_b, proj_w, proj_s,
                 proj_b):
    import ml_dtypes
    f = np.float32
    bf = ml_dtypes.bfloat16
    w = np.asarray(qkv_w, f).copy()          # [768, 256, 3, 3]
    dif = (w[:, :, 0, 1] + w[:, :, 1, 0] + w[:, :, 1, 1] + w[:, :, 1, 2]
           + w[:, :, 2, 1])
    w[:, :, 1, 1] -= THETA * dif
    w *= np.asarray(qkv_s, f)[:, None, None, None]

    # 1-D Winograd F(2,3) weight transform along kx:
    # What[0]=w0, What[1]=(w0+w1+w2)/2, What[2]=(w0-w1+w2)/2, What[3]=w2
    w0, w1, w2 = w[:, :, :, 0], w[:, :, :, 1], w[:, :, :, 2]
    wh = np.stack([w0, (w0 + w1 + w2) * 0.5, (w0 - w1 + w2) * 0.5, w2],
                  axis=0)                     # [4, 768, 256, 3(ky)]
    # WQW[mc, p, j, ky, kc, o'] = wh[j, 128*mc+o', 128*kc+p, ky]
    wh = wh.reshape(4, 6, 128, 2, 128, 3)     # [j, mc, o', kc, p, ky]
    wqw = np.ascontiguousarray(
        wh.transpose(1, 4, 0, 5, 3, 2)).astype(bf)  # [mc,p,j,ky,kc,o']

    bq = np.ascontiguousarray(np.asarray(qkv_b, f).reshape(6, 128).T)

    pe_wf = np.asarray(pe_w, f)[:, 0] * np.asarray(pe_s, f)[:, None, None]
    pew = np.zeros((128, 2, 9), f)
    for kc in range(2):
        for s in range(9):
            pew[:, kc, s] = pe_wf[128 * kc:128 * (kc + 1), s // 3, s % 3]

    pwm = np.asarray(proj_w, f)[:, :, 0, 0] * np.asarray(proj_s, f)[:, None]
    pw = np.ascontiguousarray(
        pwm.T.reshape(2, 128, 256).transpose(1, 0, 2).reshape(
            128, 512)).astype(bf)
    pbv = np.asarray(proj_b, f) + pwm @ np.asarray(pe_b, f)
    pb = np.ascontiguousarray(pbv.reshape(2, 128).T)

    return dict(wqw=wqw, bq=bq, pew=pew, pw=pw, pb=pb)


def kernel(x, qkv_w, qkv_s, qkv_b, pe_w, pe_s, pe_b, proj_w, proj_s, proj_b):
    import ml_dtypes
    from concourse.bass_utils import run_bass_kernel_spmd

    if "nc" not in _cache:
        _cache["nc"] = _build()
    nc = _cache["nc"]

    consts = _prep_consts(qkv_w, qkv_s, qkv_b, pe_w, pe_s, pe_b, proj_w,
                          proj_s, proj_b)
    x = np.asarray(x, np.float32).astype(ml_dtypes.bfloat16)
    in_maps = []
    for b in range(B):
        m = dict(consts)
        m["x"] = np.ascontiguousarray(x[b].reshape(2, 128, HW))
        in_maps.append(m)

    res = run_bass_kernel_spmd(nc, in_maps, list(range(N_CORES)), trace=False)
    out = np.empty((B, C, H, W), np.float32)
    for b in range(B):
        o = np.asarray(res.results[b]["out"], np.float32).reshape(C, HW)
        out[b] = o.reshape(C, H, W)
    return out
